# revision 1
# baseline (speedup 1.0000x reference)
"""Per-sample 21x21 blur (grouped conv, reflect pad) on trn2, 8 NeuronCores.

Problem: input [16, 3, 768, 768] f32, kernel [16, 21, 21] f32 (one blur
kernel per sample, shared across channels), reflect-pad 10, output
[16, 3, 768, 768] f32.

Strategy (data-parallel over batch, 2 samples/core, 6 images/core):
  The conv becomes TensorE matmuls via a Toeplitz factorization over image
  rows: for an output row-block of M rows, the M+20 input rows covering it
  are contracted against a banded [M+20, M] matrix T_dx holding kernel
  column dx on its diagonals; the 21 dx terms accumulate in one PSUM tile
  with the moving operand shifted along the free (column) axis by dx:

    out[y0+m, x0+n] = sum_dx  T_dx[r, m] * pad[y0+r, x0+dx+n]

  PE cost is purely streamed moving columns (1 bf16 col/cycle), i.e.
  21 * 768 columns per row-block set, so the row-block count is what
  matters.  M=108 (K=128, the partition limit) gives 7 full blocks per
  768-row image; the six 12-row remainder strips are packed into 2
  extra block-diagonal sets (4 images + 2 images stacked on partitions),
  for 44 sets/core instead of 48 with uniform M=96.

  Inputs and Toeplitz weights are pre-cast to bf16 on the host (PSUM
  accumulation stays fp32), which keeps the PE on its fast streaming path.
"""
import sys

sys.path.insert(0, "/opt/trn_rl_repo")

import numpy as np
import ml_dtypes

N_CORES = 8
B, C, H, W = 16, 3, 768, 768
KS = 21          # kernel size
PAD = 10         # reflect pad
HP = H + 2 * PAD  # 788
WP = W + 2 * PAD  # 788
MBLK = 108       # output rows per main matmul block
KBLK = 128       # input rows per main block (= partition limit)
YBLKS = H // MBLK  # 7 full blocks per image
MREM = H - YBLKS * MBLK  # 12 remainder rows per image
KREM = MREM + KS - 1     # 32 input rows per remainder strip
NBLK = 384       # legacy constant (timing probes); chunking below uses CHUNKS
CHUNKS = ((0, 512), (512, 256))  # (x0, width) pairs covering 768 cols
SPC = B // N_CORES  # samples per core = 2
IMGS = SPC * C      # images per core = 6
REM_GROUPS = ((0, 1, 2, 3), (4, 5))  # images packed per remainder set

_prog_cache = {}


def build_program(reps=1, loop_reps=1):
    """loop_reps>1 wraps the whole conv in a hardware For_i loop repeating it
    loop_reps times -- used only for timing (constant instruction count)."""
    import contextlib

    import concourse.bacc as bacc
    import concourse.mybir as mybir
    from concourse.tile import TileContext

    nc = bacc.Bacc(None, target_bir_lowering=False)
    x = nc.declare_dram_parameter("x", [IMGS, HP, WP], mybir.dt.bfloat16,
                                  isOutput=False)
    w = nc.declare_dram_parameter("w", [KBLK, SPC * KS, MBLK], mybir.dt.bfloat16,
                                  isOutput=False)
    wr = [
        nc.declare_dram_parameter(
            f"wr{gi}", [len(g) * KREM, KS, len(g) * MREM], mybir.dt.bfloat16,
            isOutput=False,
        )
        for gi, g in enumerate(REM_GROUPS)
    ]
    y = nc.declare_dram_parameter("y", [IMGS, H, W], mybir.dt.float32,
                                  isOutput=True)

    with TileContext(nc) as tc:
        with (
            tc.tile_pool(name="wpool", bufs=1) as wpool,
            tc.tile_pool(name="xpool", bufs=4) as xpool,
            tc.tile_pool(name="opool", bufs=3) as opool,
            tc.tile_pool(name="psum", bufs=8, space="PSUM") as psum_pool,
        ):
            w_sb = wpool.tile([KBLK, SPC * KS, MBLK], mybir.dt.bfloat16)
            nc.sync.dma_start(out=w_sb[:, :, :], in_=w[:, :, :])
            wr_sb = []
            for gi, g in enumerate(REM_GROUPS):
                t = wpool.tile([len(g) * KREM, KS, len(g) * MREM],
                               mybir.dt.bfloat16, tag=f"wr{gi}")
                nc.sync.dma_start(out=t[:, :, :], in_=wr[gi][:, :, :])
                wr_sb.append(t)

            loop_cm = (
                tc.For_i(0, loop_reps, 1) if loop_reps > 1
                else contextlib.nullcontext()
            )
            with loop_cm:
                for _ in range(reps):
                    # main blocks: M=108, K=128
                    for img in range(IMGS):
                        s = img // C
                        for yb in range(YBLKS):
                            x_sb = xpool.tile([KBLK, WP], mybir.dt.bfloat16)
                            nc.sync.dma_start(
                                out=x_sb[:, :],
                                in_=x[img, yb * MBLK : yb * MBLK + KBLK, :],
                            )
                            out_sb = opool.tile([MBLK, W], mybir.dt.float32)
                            for x0, wdt in CHUNKS:
                                ps = psum_pool.tile([MBLK, 512],
                                                    mybir.dt.float32)
                                for dx in range(KS):
                                    nc.tensor.matmul(
                                        ps[:, :wdt],
                                        w_sb[:, s * KS + dx, :],
                                        x_sb[:, x0 + dx : x0 + dx + wdt],
                                        start=(dx == 0),
                                        stop=(dx == KS - 1),
                                    )
                                nc.vector.tensor_copy(
                                    out=out_sb[:, x0 : x0 + wdt], in_=ps[:, :wdt]
                                )
                            nc.sync.dma_start(
                                out=y[img, yb * MBLK : (yb + 1) * MBLK, :],
                                in_=out_sb[:, :],
                            )
                    # remainder strips: images packed on partitions
                    for gi, g in enumerate(REM_GROUPS):
                        ng = len(g)
                        xr_sb = xpool.tile([ng * KREM, WP], mybir.dt.bfloat16,
                                           tag=f"xr{gi}")
                        for i, img in enumerate(g):
                            nc.sync.dma_start(
                                out=xr_sb[i * KREM : (i + 1) * KREM, :],
                                in_=x[img, YBLKS * MBLK :, :],
                            )
                        outr_sb = opool.tile([ng * MREM, W], mybir.dt.float32,
                                             tag=f"or{gi}")
                        for x0, wdt in CHUNKS:
                            ps = psum_pool.tile([ng * MREM, 512],
                                                mybir.dt.float32, tag="ps")
                            for dx in range(KS):
                                nc.tensor.matmul(
                                    ps[:, :wdt],
                                    wr_sb[gi][:, dx, :],
                                    xr_sb[:, x0 + dx : x0 + dx + wdt],
                                    start=(dx == 0),
                                    stop=(dx == KS - 1),
                                )
                            nc.vector.tensor_copy(
                                out=outr_sb[:, x0 : x0 + wdt], in_=ps[:, :wdt]
                            )
                        for i, img in enumerate(g):
                            nc.sync.dma_start(
                                out=y[img, YBLKS * MBLK :, :],
                                in_=outr_sb[i * MREM : (i + 1) * MREM, :],
                            )
    nc.compile()
    return nc


def _band(kern_col, K, M):
    """[K, M] banded Toeplitz: T[m+j, m] = kern_col[j], j in [0,21)."""
    t = np.zeros((K, M), np.float32)
    for m in range(M):
        t[m : m + KS, m] = kern_col
    return t


def _weights(kern_pair):
    """kern_pair [SPC, 21, 21] -> (w_main, [wr per group]) in bf16."""
    wt = np.zeros((KBLK, SPC * KS, MBLK), np.float32)
    for s in range(SPC):
        for dx in range(KS):
            wt[:, s * KS + dx, :] = _band(kern_pair[s, :, dx], KBLK, MBLK)
    wrs = []
    for g in REM_GROUPS:
        ng = len(g)
        wr = np.zeros((ng * KREM, KS, ng * MREM), np.float32)
        for i, img in enumerate(g):
            s = img // C
            for dx in range(KS):
                wr[i * KREM : (i + 1) * KREM, dx,
                   i * MREM : (i + 1) * MREM] = _band(
                    kern_pair[s, :, dx], KREM, MREM)
        wrs.append(wr.astype(ml_dtypes.bfloat16))
    return wt.astype(ml_dtypes.bfloat16), wrs


def make_in_maps(inp, kern):
    pad = np.pad(inp, ((0, 0), (0, 0), (PAD, PAD), (PAD, PAD)), mode="reflect")
    pad_bf = pad.astype(ml_dtypes.bfloat16)
    in_maps = []
    for c in range(N_CORES):
        s0 = c * SPC
        x_core = pad_bf[s0 : s0 + SPC].reshape(IMGS, HP, WP)
        w_core, wr_core = _weights(kern[s0 : s0 + SPC])
        m = {"x": np.ascontiguousarray(x_core), "w": w_core}
        for gi, wr in enumerate(wr_core):
            m[f"wr{gi}"] = wr
        in_maps.append(m)
    return in_maps


def kernel(input, kernel):
    from concourse.bass_utils import run_bass_kernel_spmd

    inp = np.asarray(input, dtype=np.float32)
    kern = np.asarray(kernel, dtype=np.float32)
    in_maps = make_in_maps(inp, kern)

    if "nc" not in _prog_cache:
        _prog_cache["nc"] = build_program()
    nc = _prog_cache["nc"]

    res = run_bass_kernel_spmd(nc, in_maps, list(range(N_CORES)))
    out = np.empty((B, C, H, W), np.float32)
    for c in range(N_CORES):
        out[c * SPC : (c + 1) * SPC] = res.results[c]["y"].reshape(SPC, C, H, W)
    return out



# revision 8
# speedup vs baseline: 1.0082x; 1.0082x over previous
"""Per-sample 21x21 blur (grouped conv, reflect pad) on trn2, 8 NeuronCores.

Problem: input [16, 3, 768, 768] f32, kernel [16, 21, 21] f32 (one blur
kernel per sample, shared across channels), reflect-pad 10, output
[16, 3, 768, 768] f32.

Strategy (data-parallel over batch, 2 samples/core, 6 images/core):
  The conv becomes TensorE matmuls via a Toeplitz factorization over image
  rows: for an output row-block of M rows, the M+20 input rows covering it
  are contracted against a banded [M+20, M] matrix T_dx holding kernel
  column dx on its diagonals; the 21 dx terms accumulate in one PSUM tile
  with the moving operand shifted along the free (column) axis by dx:

    out[y0+m, x0+n] = sum_dx  T_dx[r, m] * pad[y0+r, x0+dx+n]

  PE cost is purely streamed moving columns (1 bf16 col/cycle at 2.4 GHz),
  i.e. 21 * 768 columns per row-block set; M=108 (K=128 partitions) gives
  7 full blocks per 768-row image plus packed remainder strips: 44 sets
  per core, ~296 us of pure streaming.

  v2: the LDWEIGHTS instruction (~P/1.2 ns for P=108 stationary columns)
  that precedes every matmul is only partially hidden by the PE's weight
  pull-ahead, costing ~35-45 ns per matmul.  To amortize it, row blocks
  are processed in groups of 4 (resp. 3) with the dx loop OUTERMOST, so
  8 (resp. 6) consecutive matmuls share one stationary, and a post-pass
  (_dedupe_ldweights) strips the redundant InstLdweights that
  tile_legalize emits per matmul, leaving one weight load per run.

  Inputs and Toeplitz weights are pre-cast to bf16 on the host (PSUM
  accumulation stays fp32), which keeps the PE on its fast streaming path.
"""
import sys

sys.path.insert(0, "/opt/trn_rl_repo")

import numpy as np
import ml_dtypes

N_CORES = 8
B, C, H, W = 16, 3, 768, 768
KS = 21          # kernel size
PAD = 10         # reflect pad
HP = H + 2 * PAD  # 788
WP = W + 2 * PAD  # 788
MBLK = 108       # output rows per main matmul block
KBLK = 128       # input rows per main block (= partition limit)
YBLKS = H // MBLK  # 7 full blocks per image
MREM = H - YBLKS * MBLK  # 12 remainder rows per image
KREM = MREM + KS - 1     # 32 input rows per remainder strip
CHUNKS = ((0, 512), (512, 256))  # (x0, width) pairs covering 768 cols
SPC = B // N_CORES  # samples per core = 2
IMGS = SPC * C      # images per core = 6
REM_GROUPS = ((0, 1, 2, 3), (4, 5))  # images packed per remainder set
YB_GROUPS = ((0, 1, 2, 3), (4, 5, 6))  # row-block groups sharing ldweights

_prog_cache = {}


def _dedupe_ldweights(nc):
    """Remove InstLdweights whose weight AP matches the weights already
    loaded by the previous InstLdweights in the same block's PE stream.

    tile_legalize emits one InstLdweights per matmul even when consecutive
    matmuls share a stationary; the PE array keeps its weights across
    matmuls, so the reloads are redundant (~P/1.2 ns each, only partially
    hidden by the weight-load pull-ahead).  Only sync-free Ldweights are
    dropped (waits/updates stay in the stream); tracking resets at block
    boundaries and on any other PE instruction.

    The weight tiles here are written once by the startup DMA and never
    rewritten, so an elided reload can never observe stale data."""
    import concourse.mybir as mybir

    removed = 0
    for fn in nc.m.functions:
        for blk in fn.blocks:
            cur_sig = None
            keep = []
            for inst in blk.instructions:
                if getattr(inst, "engine", None) != mybir.EngineType.PE:
                    keep.append(inst)
                    continue
                if isinstance(inst, mybir.InstLdweights):
                    sig = (
                        str(inst.ins[0]),
                        str(getattr(inst, "perf_mode", None)),
                        str(getattr(inst, "is_transpose", None)),
                        str(getattr(inst, "tile_position", None)),
                    )
                    si = inst.sync_info
                    clean = si is None or (
                        len(si.on_wait) == 0 and len(si.on_update) == 0
                    )
                    if sig == cur_sig and clean:
                        removed += 1
                        continue
                    cur_sig = sig
                    keep.append(inst)
                elif isinstance(inst, mybir.InstMatmult):
                    keep.append(inst)
                else:
                    cur_sig = None
                    keep.append(inst)
            if len(keep) != len(blk.instructions):
                blk.instructions[:] = keep
    return removed


def build_program(reps=1, loop_reps=1):
    """loop_reps>1 wraps the whole conv in a hardware For_i loop repeating it
    loop_reps times -- used only for timing (constant instruction count)."""
    import contextlib

    import concourse.bacc as bacc
    import concourse.mybir as mybir
    from concourse.tile import TileContext

    nc = bacc.Bacc(None, target_bir_lowering=False)
    x = nc.declare_dram_parameter("x", [IMGS, HP, WP], mybir.dt.bfloat16,
                                  isOutput=False)
    w = nc.declare_dram_parameter("w", [KBLK, SPC * KS, MBLK], mybir.dt.bfloat16,
                                  isOutput=False)
    wr = [
        nc.declare_dram_parameter(
            f"wr{gi}", [len(g) * KREM, KS, len(g) * MREM], mybir.dt.bfloat16,
            isOutput=False,
        )
        for gi, g in enumerate(REM_GROUPS)
    ]
    y = nc.declare_dram_parameter("y", [IMGS, H, W], mybir.dt.float32,
                                  isOutput=True)

    with TileContext(nc) as tc:
        with (
            tc.tile_pool(name="wpool", bufs=1) as wpool,
            tc.tile_pool(name="xpool", bufs=8) as xpool,
            tc.tile_pool(name="opool", bufs=6) as opool,
            tc.tile_pool(name="psum", bufs=8, space="PSUM") as psum_pool,
        ):
            w_sb = wpool.tile([KBLK, SPC * KS, MBLK], mybir.dt.bfloat16)
            nc.sync.dma_start(out=w_sb[:, :, :], in_=w[:, :, :])
            wr_sb = []
            for gi, g in enumerate(REM_GROUPS):
                t = wpool.tile([len(g) * KREM, KS, len(g) * MREM],
                               mybir.dt.bfloat16, tag=f"wr{gi}")
                nc.sync.dma_start(out=t[:, :, :], in_=wr[gi][:, :, :])
                wr_sb.append(t)

            loop_cm = (
                tc.For_i(0, loop_reps, 1) if loop_reps > 1
                else contextlib.nullcontext()
            )
            with loop_cm:
                for _ in range(reps):
                    # main blocks: M=108, K=128, dx outermost within a
                    # group of row blocks so consecutive matmuls share
                    # one stationary (ldw-opt elides the reloads)
                    for img in range(IMGS):
                        s = img // C
                        for grp in YB_GROUPS:
                            xs = []
                            for yb in grp:
                                x_sb = xpool.tile([KBLK, WP],
                                                  mybir.dt.bfloat16,
                                                  tag="x_sb")
                                nc.sync.dma_start(
                                    out=x_sb[:, :],
                                    in_=x[img, yb * MBLK : yb * MBLK + KBLK, :],
                                )
                                xs.append(x_sb)
                            pss = []
                            for _yb in grp:
                                ps_a = psum_pool.tile(
                                    [MBLK, 512], mybir.dt.float32, tag="ps")
                                ps_b = psum_pool.tile(
                                    [MBLK, 512], mybir.dt.float32, tag="ps")
                                pss.append((ps_a, ps_b))
                            for dx in range(KS):
                                wap = w_sb[:, s * KS + dx, :]
                                for x_sb, (ps_a, ps_b) in zip(xs, pss):
                                    nc.tensor.matmul(
                                        ps_a[:, :512],
                                        wap,
                                        x_sb[:, dx : dx + 512],
                                        start=(dx == 0),
                                        stop=(dx == KS - 1),
                                    )
                                    nc.tensor.matmul(
                                        ps_b[:, :256],
                                        wap,
                                        x_sb[:, 512 + dx : 768 + dx],
                                        start=(dx == 0),
                                        stop=(dx == KS - 1),
                                    )
                            for yb, (ps_a, ps_b) in zip(grp, pss):
                                out_sb = opool.tile([MBLK, W],
                                                    mybir.dt.float32,
                                                    tag="out_sb")
                                nc.vector.tensor_copy(
                                    out=out_sb[:, 0:512], in_=ps_a[:, :512]
                                )
                                nc.vector.tensor_copy(
                                    out=out_sb[:, 512:768], in_=ps_b[:, :256]
                                )
                                nc.sync.dma_start(
                                    out=y[img, yb * MBLK : (yb + 1) * MBLK, :],
                                    in_=out_sb[:, :],
                                )
                    # remainder strips: images packed on partitions,
                    # dx outer, both width-chunks inner per stationary
                    for gi, g in enumerate(REM_GROUPS):
                        ng = len(g)
                        xr_sb = xpool.tile([ng * KREM, WP], mybir.dt.bfloat16,
                                           tag=f"xr{gi}")
                        for i, img in enumerate(g):
                            nc.sync.dma_start(
                                out=xr_sb[i * KREM : (i + 1) * KREM, :],
                                in_=x[img, YBLKS * MBLK :, :],
                            )
                        ps_a = psum_pool.tile([ng * MREM, 512],
                                              mybir.dt.float32, tag="ps")
                        ps_b = psum_pool.tile([ng * MREM, 512],
                                              mybir.dt.float32, tag="ps")
                        for dx in range(KS):
                            wap = wr_sb[gi][:, dx, :]
                            nc.tensor.matmul(
                                ps_a[:, :512],
                                wap,
                                xr_sb[:, dx : dx + 512],
                                start=(dx == 0),
                                stop=(dx == KS - 1),
                            )
                            nc.tensor.matmul(
                                ps_b[:, :256],
                                wap,
                                xr_sb[:, 512 + dx : 768 + dx],
                                start=(dx == 0),
                                stop=(dx == KS - 1),
                            )
                        outr_sb = opool.tile([ng * MREM, W], mybir.dt.float32,
                                             tag=f"or{gi}")
                        nc.vector.tensor_copy(
                            out=outr_sb[:, 0:512], in_=ps_a[:, :512]
                        )
                        nc.vector.tensor_copy(
                            out=outr_sb[:, 512:768], in_=ps_b[:, :256]
                        )
                        for i, img in enumerate(g):
                            nc.sync.dma_start(
                                out=y[img, YBLKS * MBLK :, :],
                                in_=outr_sb[i * MREM : (i + 1) * MREM, :],
                            )
    nc.compile()
    _dedupe_ldweights(nc)
    return nc


def _band(kern_col, K, M):
    """[K, M] banded Toeplitz: T[m+j, m] = kern_col[j], j in [0,21)."""
    t = np.zeros((K, M), np.float32)
    for m in range(M):
        t[m : m + KS, m] = kern_col
    return t


def _weights(kern_pair):
    """kern_pair [SPC, 21, 21] -> (w_main, [wr per group]) in bf16."""
    wt = np.zeros((KBLK, SPC * KS, MBLK), np.float32)
    for s in range(SPC):
        for dx in range(KS):
            wt[:, s * KS + dx, :] = _band(kern_pair[s, :, dx], KBLK, MBLK)
    wrs = []
    for g in REM_GROUPS:
        ng = len(g)
        wr = np.zeros((ng * KREM, KS, ng * MREM), np.float32)
        for i, img in enumerate(g):
            s = img // C
            for dx in range(KS):
                wr[i * KREM : (i + 1) * KREM, dx,
                   i * MREM : (i + 1) * MREM] = _band(
                    kern_pair[s, :, dx], KREM, MREM)
        wrs.append(wr.astype(ml_dtypes.bfloat16))
    return wt.astype(ml_dtypes.bfloat16), wrs


def make_in_maps(inp, kern):
    pad = np.pad(inp, ((0, 0), (0, 0), (PAD, PAD), (PAD, PAD)), mode="reflect")
    pad_bf = pad.astype(ml_dtypes.bfloat16)
    in_maps = []
    for c in range(N_CORES):
        s0 = c * SPC
        x_core = pad_bf[s0 : s0 + SPC].reshape(IMGS, HP, WP)
        w_core, wr_core = _weights(kern[s0 : s0 + SPC])
        m = {"x": np.ascontiguousarray(x_core), "w": w_core}
        for gi, wr in enumerate(wr_core):
            m[f"wr{gi}"] = wr
        in_maps.append(m)
    return in_maps


def kernel(input, kernel):
    from concourse.bass_utils import run_bass_kernel_spmd

    inp = np.asarray(input, dtype=np.float32)
    kern = np.asarray(kernel, dtype=np.float32)
    in_maps = make_in_maps(inp, kern)

    if "nc" not in _prog_cache:
        _prog_cache["nc"] = build_program()
    nc = _prog_cache["nc"]

    res = run_bass_kernel_spmd(nc, in_maps, list(range(N_CORES)))
    out = np.empty((B, C, H, W), np.float32)
    for c in range(N_CORES):
        out[c * SPC : (c + 1) * SPC] = res.results[c]["y"].reshape(SPC, C, H, W)
    return out
